# revision 11
# baseline (speedup 1.0000x reference)
"""DeepseekMoE Trainium2 kernel: expert-parallel sparse MoE across 8 NeuronCores.

Strategy:
  - Host computes routing ONLY to choose static per-slot capacities and the
    expert-piece -> (core, slot) assignment (compile-time shape decisions).
  - The device kernel recomputes the full routing math (router matmul in fp32,
    sigmoid, grouped top-k, combine weights) with vector ops batched across
    all 8 token tiles to keep the dependency chain short, builds one-hot
    token-selection matrices on-device, gathers tokens per expert-piece via
    matmuls, runs gate/up/down GEMMs in bf16, then runs the shared-expert
    gate/up, and finally walks the down projection in 512-column output
    chunks: each chunk's combine matmul feeds a chunked ReduceScatter that
    overlaps the next chunk's down GEMMs.
  - Each core returns a [128, 2048] shard; the host concatenates.
"""

import os
import sys

for _p in ("/opt/trn_rl_repo", "/root/.axon_site/_ro/trn_rl_repo"):
    if os.path.isdir(_p) and _p not in sys.path:
        sys.path.append(_p)

import numpy as np
import ml_dtypes

import concourse.bass as bass
import concourse.mybir as mybir
import concourse.tile as tile
from concourse import bacc
from concourse.bass_utils import run_bass_kernel_spmd
from concourse.masks import make_identity

P = 128
T = 1024
H = 2048
E = 32
TOPK = 6
G = 8
TOPK_G = 4
EG = E // G
MSZ = 1408          # moe_intermediate_size
NCORES = 8
NSLOT = 5           # expert-piece slots per core
NT = T // P         # 8 token tiles
NKH = H // P        # 16 hidden k-tiles
NMT = MSZ // P      # 11 m-tiles per expert (gate or up)
KG = 4              # k-tiles per weight DMA batch
SHM_PAD = 384       # padded per-core shared intermediate (352 -> 384)
SHK = SHM_PAD // P  # 3 shared k-tiles
SH_SLICE = 352      # actual per-core shared intermediate
MARGIN = 8          # per-piece capacity safety margin
NPRE = 3            # prefetched gate/up weight bands
BIG = 1.0e30

FP32 = mybir.dt.float32
FP16 = mybir.dt.float16
BF16 = mybir.dt.bfloat16
AF = mybir.ActivationFunctionType
ALU = mybir.AluOpType
AX = mybir.AxisListType


# ---------------------------------------------------------------- host routing

def host_routing(xf, w_router, corr_bias):
    logits = xf @ w_router
    scores = 1.0 / (1.0 + np.exp(-logits))
    sfc = scores + corr_bias
    grp = sfc.reshape(T, G, E // G)
    top2 = np.sort(grp, axis=-1)[..., -2:]
    gs = top2.sum(-1)
    gidx = np.argsort(-gs, axis=1)[:, :TOPK_G]
    gmask = np.zeros((T, G), bool)
    np.put_along_axis(gmask, gidx, True, axis=1)
    masked = np.where(np.repeat(gmask, E // G, axis=1), sfc, -np.inf)
    topk_idx = np.argsort(-masked, axis=1)[:, :TOPK]
    mask = np.zeros((T, E), np.float32)
    np.put_along_axis(mask, topk_idx, 1.0, axis=1)
    loads = mask.sum(0).astype(np.int64)
    return loads


def _pack(caps, loads):
    """Best-fit-decreasing fluid pack of expert loads into 8 copies of each
    cap class. Returns assign[core][slot] = (expert, r0, r1) or None."""
    bins = []
    for j, c in enumerate(caps):
        for i in range(NCORES):
            bins.append([c, i, j])
    assign = [[None] * len(caps) for _ in range(NCORES)]
    order = sorted(range(E), key=lambda e: -loads[e])
    for e in order:
        need = int(loads[e])
        r0 = 0
        while need > 0:
            fit = [b for b in bins if b[0] - MARGIN >= need]
            if fit:
                b = min(fit, key=lambda b: b[0])
                take = need
            else:
                if not bins:
                    return None
                b = max(bins, key=lambda b: b[0])
                take = b[0] - MARGIN
                if take <= 0:
                    return None
            bins.remove(b)
            is_final = (r0 + take == int(loads[e]))
            r1 = 1.0e9 if is_final else float(r0 + take)
            assign[b[1]][b[2]] = (e, r0, r1)
            r0 += take
            need -= take
    if bins:
        return None  # empty slots waste full-capacity compute
    return assign


def plan_assignment(loads):
    """Search cap vectors (NSLOT slots, multiples of 32) minimizing modeled
    PE cost; return (caps, assign)."""
    import itertools
    best = None
    for caps in itertools.combinations_with_replacement(
            range(384, 63, -32), NSLOT):
        if sum(caps) * NCORES < int(np.sum(loads)) + E * MARGIN:
            continue
        assign = _pack(caps, loads)
        if assign is None:
            continue
        gu = sum(352 * max(0.54 * c, 100.0) for c in caps)
        dn = sum(44 * 277 * ((c + 127) // 128) for c in caps)
        cost = gu + dn + 138.0 * sum(caps)
        if best is None or cost < best[0]:
            best = (cost, caps, assign)
    if best is None:  # fall back: one huge slot vector
        caps = tuple([((int(max(loads)) + MARGIN + 31) // 32) * 32] * NSLOT)
        assign = _pack(caps, loads)
        best = (0.0, caps, assign)
    return best[1], best[2]


# ---------------------------------------------------------------- device build

def legal_span(b):
    # max partition count addressable from base b (HW quadrant rule)
    return 128 if b == 0 else 64 if b == 64 else 32


def row_segments(lo, hi):
    """Split [lo, hi) (multiples of 32 within a 128 tile) into quadrant-legal
    segments."""
    segs = []
    while lo < hi:
        cnt = min(hi - lo, legal_span(lo))
        segs.append((lo, cnt))
        lo += cnt
    return segs


def chunks_of(width, step=512):
    out = []
    lo = 0
    while lo < width:
        out.append((lo, min(lo + step, width)))
        lo += step
    return out


def build_kernel(caps):
    CT = sum(caps)
    COFF = [sum(caps[:j]) for j in range(NSLOT)]
    CMAX = max(caps)
    CTN = [(c + P - 1) // P for c in caps]   # c-tiles per slot
    NGCT = (CT + P - 1) // P

    nc = bacc.Bacc("TRN2", target_bir_lowering=False)

    # -------- DRAM I/O (per core)
    x_bf = nc.dram_tensor("x_bf", [T, H], BF16, kind="ExternalInput")
    xT_f32 = nc.dram_tensor("xT_f32", [H, T], FP32, kind="ExternalInput")
    xT_bf = nc.dram_tensor("xT_bf", [H, T], BF16, kind="ExternalInput")
    w_router = nc.dram_tensor("w_router", [H, E], FP32, kind="ExternalInput")
    cbias_rep = nc.dram_tensor("cbias_rep", [P, E], FP32,
                               kind="ExternalInput")
    # esel replicated 3x along partitions (for lhsT bases 0/32/64)
    esel = nc.dram_tensor("esel", [3 * E, NSLOT], FP32,
                          kind="ExternalInput")
    r0s = nc.dram_tensor("r0s", [P, NSLOT], FP32, kind="ExternalInput")
    r1s = nc.dram_tensor("r1s", [P, NSLOT], FP32, kind="ExternalInput")
    # gate/up paired bands: [slot, band(11), kgroup(4), P, KG, 256]
    wgu = nc.dram_tensor(
        "wgu", [NSLOT, NMT, NKH // KG, P, KG, 2 * P], BF16,
        kind="ExternalInput")
    # down: [slot, nchunk(4), kt(11), P, 512]
    wdn = nc.dram_tensor(
        "wdn", [NSLOT, 4, NMT, P, 512], BF16, kind="ExternalInput")
    # shared gate/up: [kt(16), P, band(3), 256]  (gate|up in last dim)
    wsgu = nc.dram_tensor(
        "wsgu", [NKH, P, SHK, 2 * P], BF16, kind="ExternalInput")
    # shared down: [kt(3), P, 2048]
    wsdn = nc.dram_tensor("wsdn", [SHK, P, H], BF16, kind="ExternalInput")
    out_shard = nc.dram_tensor("out_shard", [P, H], FP32,
                               kind="ExternalOutput")

    with tile.TileContext(nc) as tc:
        with (
            tc.tile_pool(name="const", bufs=1) as const,
            tc.tile_pool(name="persist", bufs=1) as persist,
            tc.tile_pool(name="dram", bufs=1, space="DRAM") as dram,
        ):
            # ---------------- constants (sync queue, tiny, before weights)
            ident_f = const.tile([P, P], FP32)
            make_identity(nc, ident_f)
            ident_b = const.tile([P, P], BF16)
            make_identity(nc, ident_b)
            iota_i = const.tile([P, CMAX], mybir.dt.int32)
            nc.gpsimd.iota(iota_i[:], pattern=[[1, CMAX]], base=0,
                           channel_multiplier=0)
            iota_f = const.tile([P, CMAX], FP32)
            nc.vector.tensor_copy(iota_f[:], iota_i[:])

            cbias_sb = const.tile([P, E], FP32)
            nc.sync.dma_start(cbias_sb[:], cbias_rep[:])
            esel_sb = const.tile([3 * E, NSLOT], FP32)
            nc.sync.dma_start(esel_sb[:], esel[:])
            r0_sb = const.tile([P, NSLOT], FP32)
            nc.sync.dma_start(r0_sb[:], r0s[:])
            r1_sb = const.tile([P, NSLOT], FP32)
            nc.sync.dma_start(r1_sb[:], r1s[:])

            # prefetch tiles for the first gate/up weight bands (sync queue:
            # this queue carries only expert-weight streams so nothing blocks
            # them)
            wgupre = []
            for pi in range(NPRE):
                t_ = persist.tile([P, NKH // KG, KG, 2 * P], BF16,
                                  tag=f"wgupre{pi}", name=f"wgupre{pi}")
                nc.sync.dma_start(
                    t_[:], wgu[0, pi].rearrange("kg p k n -> p kg k n"))
                wgupre.append(t_)

            # persistent intermediates
            z_sb = persist.tile([P, NT, 3 * NSLOT], FP32)
            zadj = persist.tile([P, NT, NSLOT], FP32)
            zgm = persist.tile([P, NT, NSLOT], FP32)   # mask gated by r1
            zgw = persist.tile([P, NT, NSLOT], FP32)   # weight gated by r1
            shact = persist.tile([P, SHK, T], BF16)
            act = persist.tile([P, NMT, CT], BF16)     # all slots
            wselall = persist.tile([P, NGCT, T], BF16)
            wsdn_sb = []
            for sk in range(SHK):
                t_ = persist.tile([P, H], BF16, tag=f"wsdn{sk}",
                                  name=f"wsdn{sk}")
                nc.gpsimd.dma_start(t_[:], wsdn[sk])
                wsdn_sb.append(t_)

            # ================ routing phase (scoped) ================
            with (
                tc.tile_pool(name="routA", bufs=1) as routA,
                tc.tile_pool(name="xf32p", bufs=2) as xf32p,
                tc.tile_pool(name="small", bufs=1) as small,
                tc.tile_pool(name="psR", bufs=1, space="PSUM") as psR,
                tc.tile_pool(name="psT", bufs=2, space="PSUM") as psT,
            ):
                wr_sb = routA.tile([P, NKH, E], FP32)
                nc.gpsimd.dma_start(
                    wr_sb[:],
                    w_router.ap().rearrange("(kt p) e -> p kt e", p=P))
                lg_ps = psR.tile([E, T], FP32)
                for kt in range(NKH):
                    xf_t = xf32p.tile([P, T], FP32, tag="xf32", name="xf32")
                    nc.gpsimd.dma_start(xf_t[:],
                                        xT_f32[kt * P:(kt + 1) * P, :])
                    for hh in range(2):
                        nc.tensor.matmul(
                            lg_ps[:, hh * 512:(hh + 1) * 512],
                            wr_sb[:, kt, :],
                            xf_t[:, hh * 512:(hh + 1) * 512],
                            start=(kt == 0), stop=(kt == NKH - 1))
                scoresT = routA.tile([E, T], FP32)
                # sigmoid via exp + reciprocal: tracks the fp32 reference
                # to ~2e-7 (the ACT Sigmoid table is only ~1e-6 accurate,
                # which flips near-tie top-k choices)
                e_sb = routA.tile([E, T], FP32)
                nc.scalar.activation(e_sb[:], lg_ps[:], AF.Exp, scale=-1.0)
                nc.vector.tensor_single_scalar(e_sb[:], e_sb[:], 1.0, ALU.add)
                nc.vector.reciprocal(scoresT[:], e_sb[:])

                # transpose scores to [T, E] tiles
                sco_tl = routA.tile([P, NT, E], FP32)
                for tt in range(NT):
                    ps = psT.tile([P, P], FP32, tag="tpz", name="tp1")
                    nc.tensor.transpose(
                        ps[:, :E], scoresT[:, tt * P:(tt + 1) * P],
                        ident_f[:E, :E])
                    nc.vector.tensor_copy(sco_tl[:, tt, :], ps[:, :E])

                # ---- batched grouped top-k routing across all token tiles
                sfc_tl = routA.tile([P, NT, E], FP32)
                cb_b = cbias_sb[:].unsqueeze(1).broadcast_to((P, NT, E))
                nc.vector.tensor_tensor(sfc_tl[:], sco_tl[:], cb_b, ALU.add)
                grp = sfc_tl[:].rearrange("p t (g k) -> p t g k", k=EG)
                max1 = small.tile([P, NT, G], FP32)
                nc.vector.tensor_reduce(max1[:], grp, AX.X, ALU.max)
                m1b = max1[:].unsqueeze(-1).broadcast_to((P, NT, G, EG))
                eq = small.tile([P, NT, E], FP32)
                eqg = eq[:].rearrange("p t (g k) -> p t g k", k=EG)
                nc.vector.tensor_tensor(eqg, grp, m1b, ALU.is_equal)
                m2 = small.tile([P, NT, E], FP32)
                m2g = m2[:].rearrange("p t (g k) -> p t g k", k=EG)
                nc.vector.scalar_tensor_tensor(
                    m2g, eqg, -BIG, grp, op0=ALU.mult, op1=ALU.add)
                max2 = small.tile([P, NT, G], FP32)
                nc.vector.tensor_reduce(max2[:], m2g, AX.X, ALU.max)
                gs = small.tile([P, NT, G], FP32)
                nc.vector.tensor_tensor(gs[:], max1[:], max2[:], ALU.add)
                gs8 = small.tile([P, NT, 8], FP32)
                for tt in range(NT):
                    nc.vector.max(gs8[:, tt, :], gs[:, tt, :])
                g8b = gs8[:, :, TOPK_G - 1:TOPK_G].broadcast_to((P, NT, G))
                gmask = small.tile([P, NT, G], FP32)
                nc.vector.tensor_tensor(gmask[:], gs[:], g8b, ALU.is_ge)
                pen = small.tile([P, NT, G], FP32)
                nc.vector.tensor_scalar(
                    pen[:], gmask[:], BIG, BIG,
                    op0=ALU.mult, op1=ALU.subtract)
                penb = pen[:].unsqueeze(-1).broadcast_to((P, NT, G, EG))
                mskd = small.tile([P, NT, E], FP32)
                mskdg = mskd[:].rearrange("p t (g k) -> p t g k", k=EG)
                nc.vector.tensor_tensor(mskdg, grp, penb, ALU.add)
                ms8 = small.tile([P, NT, 8], FP32)
                for tt in range(NT):
                    nc.vector.max(ms8[:, tt, :], mskd[:, tt, :])
                m8b = ms8[:, :, TOPK - 1:TOPK].broadcast_to((P, NT, E))
                cmask = small.tile([P, NT, E], FP32)
                nc.vector.tensor_tensor(cmask[:], mskd[:], m8b, ALU.is_ge)
                comb_tl = routA.tile([P, NT, E], FP32)
                nc.vector.tensor_tensor(
                    comb_tl[:], sco_tl[:], cmask[:], ALU.mult)

                # combT / maskT / rank
                combT = routA.tile([E, T], FP32)
                for tt in range(NT):
                    ps = psT.tile([P, P], FP32, tag="tpz", name="tpc")
                    nc.tensor.transpose(
                        ps[:E, :], comb_tl[:, tt, :], ident_f[:, :])
                    nc.vector.tensor_copy(
                        combT[:, tt * P:(tt + 1) * P], ps[:E, :])
                maskT = routA.tile([E, T], FP32)
                nc.vector.tensor_single_scalar(
                    maskT[:], combT[:], 0.0, ALU.is_gt)
                rankT = routA.tile([E, T], FP32)
                nc.vector.tensor_tensor_scan(
                    rankT[:], maskT[:], maskT[:], 0.0,
                    op0=ALU.add, op1=ALU.bypass)
                rank0T = routA.tile([E, T], FP32)
                nc.vector.tensor_tensor(
                    rank0T[:], rankT[:], maskT[:], ALU.subtract)

                # per-slot columns: z = [rank0 | mask | w] per token
                for tt in range(NT):
                    zps = psT.tile([P, P], FP32, tag="tpz", name="zps")
                    for q, src in enumerate((rank0T, maskT, combT)):
                        nc.tensor.matmul(
                            zps[:, q * NSLOT:(q + 1) * NSLOT],
                            src[:, tt * P:(tt + 1) * P],
                            esel_sb[0:E, :],
                            start=True, stop=True)
                    nc.vector.tensor_copy(
                        z_sb[:, tt, :], zps[:, 0:3 * NSLOT])
                # rank adjusted by piece offset r0
                r0b = r0_sb[:].unsqueeze(1).broadcast_to((P, NT, NSLOT))
                nc.vector.tensor_tensor(
                    zadj[:], z_sb[:, :, 0:NSLOT], r0b, ALU.subtract)
                # gate mask/weight by rank < r1 (upper piece bound)
                r1b = r1_sb[:].unsqueeze(1).broadcast_to((P, NT, NSLOT))
                gate = small.tile([P, NT, NSLOT], FP32)
                nc.vector.tensor_tensor(
                    gate[:], z_sb[:, :, 0:NSLOT], r1b, ALU.is_lt)
                nc.vector.tensor_tensor(
                    zgm[:], z_sb[:, :, NSLOT:2 * NSLOT], gate[:], ALU.mult)
                nc.vector.tensor_tensor(
                    zgw[:], z_sb[:, :, 2 * NSLOT:3 * NSLOT], gate[:],
                    ALU.mult)

                # warm up the collective path early so the first real
                # ReduceScatter doesn't pay route-setup + core-skew costs
                ccw_in = dram.tile([1, 64], FP32, name="ccw_in")
                ccw_out = dram.tile([1, 64], FP32, name="ccw_out")
                nc.gpsimd.collective_compute(
                    "AllReduce", ALU.add,
                    replica_groups=[list(range(NCORES))],
                    ins=[ccw_in.opt()], outs=[ccw_out.opt()])

            # ================ gather + gate/up (scoped) ================
            with (
                tc.tile_pool(name="gusmall", bufs=2) as gusmall,
                tc.tile_pool(name="expbig", bufs=1) as expbig,
            ):
                xg = expbig.tile([P, NKH, CT], BF16)
                # ---- selection matrices for all slots: [t, c]
                with (
                    tc.tile_pool(name="selp", bufs=1) as selp,
                    tc.tile_pool(name="psG", bufs=1, space="PSUM") as psG,
                ):
                    selT = selp.tile([P, NT, CT], BF16)
                    for j in range(NSLOT):
                        ssl = selT[:, :, COFF[j]:COFF[j] + caps[j]]
                        iob = iota_f[:, 0:caps[j]].unsqueeze(1).broadcast_to(
                            (P, NT, caps[j]))
                        zab = zadj[:, :, j:j + 1].broadcast_to(
                            (P, NT, caps[j]))
                        zmb = zgm[:, :, j:j + 1].broadcast_to(
                            (P, NT, caps[j]))
                        nc.vector.tensor_tensor(ssl, iob, zab, ALU.is_equal)
                        nc.vector.tensor_tensor(ssl, ssl, zmb, ALU.mult)
                    # ---- gather all slots: xg[h, c] = x^T @ selT
                    gchunks = chunks_of(CT)
                    for half in range(2):
                        xbh = []
                        for tt in range(NT):
                            t_ = selp.tile([P, H // 2], BF16, tag=f"xb{tt}",
                                           name=f"xb{tt}", bufs=2)
                            nc.scalar.dma_start(
                                t_[:], x_bf[tt * P:(tt + 1) * P,
                                            half * (H // 2):
                                            (half + 1) * (H // 2)])
                            xbh.append(t_)
                        for hl in range(NKH // 2):
                            ht = half * (NKH // 2) + hl
                            gps = psG.tile([P, CT], FP32, tag="gps",
                                           name="gps")
                            for tt in range(NT):
                                for (lo, hi) in gchunks:
                                    nc.tensor.matmul(
                                        gps[:, lo:hi],
                                        xbh[tt][:, hl * P:(hl + 1) * P],
                                        selT[:, tt, lo:hi],
                                        start=(tt == 0), stop=(tt == NT - 1))
                            nc.vector.tensor_copy(
                                xg[:, ht, :], gps[:, 0:CT])

                # ---- expert gate/up GEMMs, all slots
                with (
                    tc.tile_pool(name="wstream", bufs=3) as wstream,
                    tc.tile_pool(name="wsb", bufs=2) as wsb,
                    tc.tile_pool(name="psGU", bufs=2, space="PSUM") as psGU,
                    tc.tile_pool(name="psW", bufs=2, space="PSUM") as psW,
                ):
                    for j in range(NSLOT):
                        cj = caps[j]
                        for mb in range(NMT):
                            g_ps = psGU.tile([P, CMAX], FP32, tag="gug",
                                             name="gug")
                            u_ps = psGU.tile([P, CMAX], FP32, tag="guu",
                                             name="guu")
                            if j == 0 and mb < NPRE:
                                wt = wgupre[mb]
                            else:
                                wt = wstream.tile(
                                    [P, NKH // KG, KG, 2 * P],
                                    BF16, tag="wgu", name="wgu_t")
                                nc.sync.dma_start(
                                    wt[:],
                                    wgu[j, mb].rearrange(
                                        "kg p k n -> p kg k n"))
                            for kt in range(NKH):
                                kg, k2 = divmod(kt, KG)
                                nc.tensor.matmul(
                                    g_ps[:, 0:cj], wt[:, kg, k2, 0:P],
                                    xg[:, kt, COFF[j]:COFF[j] + cj],
                                    start=(kt == 0), stop=(kt == NKH - 1))
                                nc.tensor.matmul(
                                    u_ps[:, 0:cj], wt[:, kg, k2, P:2 * P],
                                    xg[:, kt, COFF[j]:COFF[j] + cj],
                                    start=(kt == 0), stop=(kt == NKH - 1))
                            t1 = gusmall.tile([P, CMAX], FP32, tag="silu",
                                              name="silu")
                            nc.scalar.activation(
                                t1[:, 0:cj], g_ps[:, 0:cj], AF.Silu)
                            nc.vector.tensor_tensor(
                                act[:, mb, COFF[j]:COFF[j] + cj],
                                t1[:, 0:cj], u_ps[:, 0:cj], ALU.mult)
                            if j == 0 and mb == 0:
                                # W_sel tiles [c, t] (weighted one-hot):
                                # vector/PE fill in behind slot 0's stream
                                wselT = wsb.tile([P, NT, CT], BF16,
                                                 tag="wselT", name="wselT",
                                                 bufs=1)
                                for j2 in range(NSLOT):
                                    wsl = wselT[:, :,
                                                COFF[j2]:COFF[j2] + caps[j2]]
                                    iob = iota_f[:, 0:caps[j2]].unsqueeze(
                                        1).broadcast_to((P, NT, caps[j2]))
                                    zab = zadj[:, :, j2:j2 + 1].broadcast_to(
                                        (P, NT, caps[j2]))
                                    zwb = zgw[:, :, j2:j2 + 1].broadcast_to(
                                        (P, NT, caps[j2]))
                                    nc.vector.tensor_tensor(
                                        wsl, iob, zab, ALU.is_equal)
                                    nc.vector.tensor_tensor(
                                        wsl, wsl, zwb, ALU.mult)
                                for tt in range(NT):
                                    for g in range(NGCT):
                                        w = min(P, CT - g * P)
                                        ps = psW.tile([P, P], BF16,
                                                      tag="tpw", name="tpw")
                                        nc.tensor.transpose(
                                            ps[:w, :],
                                            wselT[:, tt, g * P:g * P + w],
                                            ident_b[:, :])
                                        nc.scalar.copy(
                                            wselall[0:w, g,
                                                    tt * P:(tt + 1) * P],
                                            ps[:w, :])

            # ================ shared expert gate/up (scoped) ================
            # placed after the routed gate/up so its 6-bank PSUM use and
            # xT_bf stream don't contend with the routing/gather phases;
            # xT_bf is read exactly once.
            with (
                tc.tile_pool(name="shx", bufs=3) as shx,
                tc.tile_pool(name="shsmall", bufs=2) as shsmall,
                tc.tile_pool(name="wsgup", bufs=1) as wsgup,
                tc.tile_pool(name="psSH", bufs=1, space="PSUM") as psSH,
            ):
                wsgu_sb = wsgup.tile([P, NKH, SHK, 2 * P], BF16)
                nc.gpsimd.dma_start(
                    wsgu_sb[:], wsgu.ap().rearrange("k p s n -> p k s n"))
                for th in range(2):
                    sl = slice(th * 512, (th + 1) * 512)
                    g_ps = [psSH.tile([P, 512], FP32, tag=f"shg{b}",
                                      name=f"shg{b}") for b in range(SHK)]
                    u_ps = [psSH.tile([P, 512], FP32, tag=f"shu{b}",
                                      name=f"shu{b}") for b in range(SHK)]
                    for kt in range(NKH):
                        xtb_t = shx.tile([P, 512], BF16, tag="xtb",
                                         name="xtb_t")
                        nc.gpsimd.dma_start(
                            xtb_t[:], xT_bf[kt * P:(kt + 1) * P, sl])
                        for b in range(SHK):
                            nc.tensor.matmul(
                                g_ps[b][:], wsgu_sb[:, kt, b, 0:P],
                                xtb_t[:],
                                start=(kt == 0), stop=(kt == NKH - 1))
                            nc.tensor.matmul(
                                u_ps[b][:], wsgu_sb[:, kt, b, P:2 * P],
                                xtb_t[:],
                                start=(kt == 0), stop=(kt == NKH - 1))
                    for b in range(SHK):
                        t1 = shsmall.tile([P, 512], FP32, tag="sh_silu",
                                          name="sh_silu")
                        nc.scalar.activation(t1[:], g_ps[b][:], AF.Silu)
                        nc.vector.tensor_tensor(
                            shact[:, b, sl], t1[:], u_ps[b][:], ALU.mult)

            # ========== down + combine + ReduceScatter, 512-col chunks ======
            partial_hc = []
            rs_hc = []
            for hc in range(4):
                t_ = dram.tile([T, 512], FP16, name=f"partial{hc}")
                partial_hc.append(t_)
                t_ = dram.tile([P, 512], FP16, name=f"rs{hc}")
                rs_hc.append(t_)
            with (
                tc.tile_pool(name="dnw", bufs=3) as dnw,
                tc.tile_pool(name="dtsp", bufs=2) as dtsp,
                tc.tile_pool(name="cmb", bufs=3) as cmb,
                tc.tile_pool(name="psD", bufs=2, space="PSUM") as psD,
                tc.tile_pool(name="psO", bufs=2, space="PSUM") as psO,
            ):
                nk = NGCT + SHK
                for nch in range(4):
                    dts = dtsp.tile([P, NGCT, 512], BF16, tag="dts",
                                    name="dts")
                    for j in range(NSLOT):
                        cj = caps[j]
                        wt = dnw.tile([P, NMT, 512], BF16, tag="wdn",
                                      name="wdn_t")
                        nc.sync.dma_start(
                            wt[:],
                            wdn[j, nch].rearrange("kt p n -> p kt n"))
                        ctgroups = [list(range(CTN[j]))[k:k + 2]
                                    for k in range(0, CTN[j], 2)]
                        for ctg in ctgroups:
                            dps = {ct: psD.tile([P, 512], FP32,
                                                tag=f"dps{gi}",
                                                name=f"dps{gi}")
                                   for gi, ct in enumerate(ctg)}
                            for kt in range(NMT):
                                for ct in ctg:
                                    w = min(P, cj - ct * P)
                                    nc.tensor.matmul(
                                        dps[ct][:w, :],
                                        act[:, kt,
                                            COFF[j] + ct * P:
                                            COFF[j] + ct * P + w],
                                        wt[:, kt, :],
                                        start=(kt == 0),
                                        stop=(kt == NMT - 1))
                            for gi, ct in enumerate(ctg):
                                w = min(P, cj - ct * P)
                                glo = COFF[j] + ct * P
                                done = 0
                                while done < w:
                                    g, off = divmod(glo + done, P)
                                    cnt = min(w - done, P - off,
                                              legal_span(off),
                                              legal_span(done))
                                    nc.vector.tensor_copy(
                                        dts[off:off + cnt, g, :],
                                        dps[ct][done:done + cnt, :])
                                    done += cnt
                    # ---- combine this 512-col chunk + shared down
                    for tt in range(NT):
                        ps = psO.tile([P, 512], FP32, tag="out",
                                      name="outps")
                        ki = 0
                        for g in range(NGCT):
                            w = min(P, CT - g * P)
                            nc.tensor.matmul(
                                ps[:],
                                wselall[0:w, g, tt * P:(tt + 1) * P],
                                dts[0:w, g, :],
                                start=(ki == 0), stop=(ki == nk - 1))
                            ki += 1
                        for sk in range(SHK):
                            nc.tensor.matmul(
                                ps[:],
                                shact[:, sk, tt * P:(tt + 1) * P],
                                wsdn_sb[sk][:, nch * 512:(nch + 1) * 512],
                                start=(ki == 0), stop=(ki == nk - 1))
                            ki += 1
                        och = cmb.tile([P, 512], FP16, tag="och",
                                       name="och")
                        nc.scalar.copy(och[:], ps[:])
                        nc.gpsimd.dma_start(
                            partial_hc[nch][tt * P:(tt + 1) * P, :], och[:])
                    nc.gpsimd.collective_compute(
                        "ReduceScatter",
                        ALU.add,
                        replica_groups=[list(range(NCORES))],
                        ins=[partial_hc[nch].opt()],
                        outs=[rs_hc[nch].opt()],
                    )
                    rs_sb = cmb.tile([P, 512], FP16, tag="rs_sb",
                                     name="rs_sb")
                    nc.gpsimd.dma_start(rs_sb[:], rs_hc[nch][:])
                    rs_f32 = cmb.tile([P, 512], FP32, tag="rs_f32",
                                      name="rs_f32")
                    nc.vector.tensor_copy(rs_f32[:], rs_sb[:])
                    nc.gpsimd.dma_start(
                        out_shard[:, nch * 512:(nch + 1) * 512], rs_f32[:])

    nc.finalize()
    return nc


_KERNEL_CACHE = {}


def get_kernel(caps):
    if caps not in _KERNEL_CACHE:
        _KERNEL_CACHE[caps] = build_kernel(caps)
    return _KERNEL_CACHE[caps]


# ---------------------------------------------------------------- entry point

def prepare_inputs(xf, w_router, corr_bias, gate_w, up_w, down_w,
                   sh_gate_w, sh_up_w, sh_down_w, caps, assign):
    bf = ml_dtypes.bfloat16
    x_bf = xf.astype(bf)
    xT_f32 = np.ascontiguousarray(xf.T)
    xT_bf = xT_f32.astype(bf)
    cb_rep = np.broadcast_to(
        corr_bias.astype(np.float32), (P, E)).copy()

    in_maps = []
    for i in range(NCORES):
        wgu_i = np.zeros((NSLOT, NMT, NKH // KG, P, KG, 2 * P), dtype=bf)
        wdn_i = np.zeros((NSLOT, 4, NMT, P, 512), dtype=bf)
        esel_i = np.zeros((E, NSLOT), np.float32)
        r0_i = np.zeros((NSLOT,), np.float32)
        r1_i = np.full((NSLOT,), 1.0e9, np.float32)
        for j in range(NSLOT):
            piece = assign[i][j]
            if piece is None:
                continue
            e, r0, r1 = piece
            r1_i[j] = r1
            esel_i[e, j] = 1.0
            r0_i[j] = float(r0)
            gw = gate_w[e].reshape(NKH // KG, KG, P, NMT, P)
            uw = up_w[e].reshape(NKH // KG, KG, P, NMT, P)
            wgu_i[j, :, :, :, :, 0:P] = gw.transpose(3, 0, 2, 1, 4)
            wgu_i[j, :, :, :, :, P:2 * P] = uw.transpose(3, 0, 2, 1, 4)
            wdn_i[j] = down_w[e].reshape(NMT, P, 4, 512).transpose(2, 0, 1, 3)
        esel3 = np.concatenate([esel_i] * 3, axis=0)
        r0rep = np.broadcast_to(r0_i, (P, NSLOT)).copy()
        r1rep = np.broadcast_to(r1_i, (P, NSLOT)).copy()

        lo = i * SH_SLICE
        hi = lo + SH_SLICE
        g_sl = np.zeros((H, SHM_PAD), np.float32)
        u_sl = np.zeros((H, SHM_PAD), np.float32)
        g_sl[:, :SH_SLICE] = sh_gate_w[:, lo:hi]
        u_sl[:, :SH_SLICE] = sh_up_w[:, lo:hi]
        # [kt, p, band, gate|up]
        wsgu_i = np.zeros((NKH, P, SHK, 2 * P), dtype=bf)
        wsgu_i[:, :, :, 0:P] = g_sl.reshape(NKH, P, SHK, P)
        wsgu_i[:, :, :, P:2 * P] = u_sl.reshape(NKH, P, SHK, P)
        d_sl = np.zeros((SHM_PAD, H), np.float32)
        d_sl[:SH_SLICE] = sh_down_w[lo:hi]
        wsdn_i = d_sl.reshape(SHK, P, H).astype(bf)

        in_maps.append({
            "x_bf": x_bf,
            "xT_f32": xT_f32,
            "xT_bf": xT_bf,
            "w_router": w_router.astype(np.float32),
            "cbias_rep": cb_rep,
            "esel": esel3,
            "r0s": r0rep,
            "r1s": r1rep,
            "wgu": wgu_i,
            "wdn": wdn_i,
            "wsgu": wsgu_i,
            "wsdn": wsdn_i,
        })
    return in_maps


def kernel(x, w_router, corr_bias, gate_w, up_w, down_w,
           sh_gate_w, sh_up_w, sh_down_w):
    x = np.asarray(x, dtype=np.float32)
    w_router = np.asarray(w_router, dtype=np.float32)
    corr_bias = np.asarray(corr_bias, dtype=np.float32)
    gate_w = np.asarray(gate_w, dtype=np.float32)
    up_w = np.asarray(up_w, dtype=np.float32)
    down_w = np.asarray(down_w, dtype=np.float32)
    sh_gate_w = np.asarray(sh_gate_w, dtype=np.float32)
    sh_up_w = np.asarray(sh_up_w, dtype=np.float32)
    sh_down_w = np.asarray(sh_down_w, dtype=np.float32)

    b, s, h = x.shape
    xf = x.reshape(T, H)

    loads = host_routing(xf, w_router, corr_bias)
    caps, assign = plan_assignment(loads)
    nc = get_kernel(caps)
    in_maps = prepare_inputs(xf, w_router, corr_bias, gate_w, up_w, down_w,
                             sh_gate_w, sh_up_w, sh_down_w, caps, assign)

    res = None
    for attempt in range(3):
        try:
            res = run_bass_kernel_spmd(nc, in_maps, list(range(NCORES)))
            break
        except Exception:
            if attempt == 2:
                raise
            import time
            time.sleep(5.0)
    out = np.concatenate(
        [res.results[i]["out_shard"] for i in range(NCORES)], axis=0)
    return out.reshape(b, s, h).astype(np.float32)
